# revision 7
# baseline (speedup 1.0000x reference)
"""AdaAttN attention kernel for 8 TRN2 NeuronCores.

Problem: nn_AdaAttN_29076928593982
  fc, fs, fcs: (4, 4096, 256) f32; Wf/Wg/Wh (256,256); bf/bg/bh (256,)
  Q = Wf@inorm(fc_t)+bf; K = Wg@inorm(fs_t)+bg; V = Wh@fs_t+bh
  A = softmax(Q K); M = A V; Var = A V^2 - M^2; S = sqrt(max(Var,1e-6))
  out = S * inorm(fcs_t) + M   (all in (b, t, d))

Sharding: data-parallel over (sample, query-half): core i -> sample i//2,
query rows [ (i%2)*2048, +2048 ). K/V replicated per sample. No collectives.

Device strategy (v2 — see kernel_baseline.py for v1):
  - ONE ACT table set for the whole kernel: every ScalarE function is in
    natural_log_exp_and_others {Exp, Ln, Copy, Identity, Square}. sqrt is
    computed as exp(0.5*ln(x)); inv_std as exp(-0.5*ln(var+eps)). This
    removes the ~22 ACT_TABLE_LOADs (~59us) the v1 kernel paid.
  - softmax exp over [128,1024] tiles (4 tk-blocks per ACTIVATE) to amortize
    the 352-cycle ACT fixed overhead; psl PSUM tiles are 2 banks each.
  - K's bias AND instance-norm mean correction are dropped entirely: both
    contribute per-query constants to the logits, which cancel exactly in
    softmax. K^T evacuates as a plain copy; only the 1/std scale is folded
    into the weights.
  - V projection is interleaved into the fs stats DMA stream (V needs no
    stats), so the PE warms up ~2us in and never re-throttles.
  - fcs stats run DVE-only and the nfcs normalize runs on GpSimd, keeping
    ACT free for softmax exp during attention.
  - all matmuls f32r (full PE speed at moving>=256); logits transposed
    (tk on partitions) with a global exp shift C0.
"""
import sys

sys.path.insert(0, "/opt/trn_rl_repo")

import numpy as np

import concourse.bass as bass
import concourse.tile as tile
from concourse import bacc
from concourse import mybir
from concourse.bass_utils import run_bass_kernel_spmd

F32 = mybir.dt.float32
F32R = mybir.dt.float32r
I32 = mybir.dt.int32
AF = mybir.ActivationFunctionType
OP = mybir.AluOpType

# rsqrt bit-trick constants: MAGIC - (bits>>1) == ((bits>>1) ^ 0x7FFFFFFF) - REST
RSQRT_MAGIC = 0x5F3759DF
RSQRT_REST = 0x7FFFFFFF - RSQRT_MAGIC

P = 128          # partitions
D = 256          # feature dim
T = 4096         # tokens per sample
TH = 2048        # query tokens per core
CH = 2           # channel chunks (D // P)
NB = T // P      # tk blocks (32)
NG = NB // 4     # tk groups of 4 blocks (8)
NQ = TH // 256   # tq chunks of 256 (8)
C0 = 110.0       # global softmax shift
EPS_IN = 1e-5
EPS_VAR = 1e-6
CK = 2048        # stats DMA chunk width (1 MiB per transfer)
NCK = T // CK    # 2

TRACE = False    # test.py sets this to get exec_time_ns
TRACE_KW = {}


def _bcast_row(handle, offset, n):
    """AP reading a DRAM row of n elements broadcast across 128 partitions."""
    return bass.AP(tensor=handle, offset=offset, ap=[[0, P], [1, n]])


def build_nc():
    nc = bacc.Bacc()

    fsT = nc.declare_dram_parameter("fsT", [D, T], F32, isOutput=False)
    fcT = nc.declare_dram_parameter("fcT", [D, T], F32, isOutput=False)
    fcsT = nc.declare_dram_parameter("fcsT", [D, T], F32, isOutput=False)
    fcsh = nc.declare_dram_parameter("fcsh", [TH, D], F32, isOutput=False)
    wfT = nc.declare_dram_parameter("wfT", [D, D], F32, isOutput=False)
    wgT = nc.declare_dram_parameter("wgT", [D, D], F32, isOutput=False)
    whT = nc.declare_dram_parameter("whT", [D, D], F32, isOutput=False)
    bq_e = nc.declare_dram_parameter("bq", [D, 1], F32, isOutput=False)
    bv_e = nc.declare_dram_parameter("bv", [D], F32, isOutput=False)
    out_e = nc.declare_dram_parameter("out", [TH, D], F32, isOutput=True)

    scm = nc.dram_tensor("scm", [2, D], F32)  # fcs stats roundtrip scratch

    with tile.TileContext(nc) as tc:
        persist = tc.tile_pool(name="persist", bufs=1)
        pp = persist.__enter__()

        QTr = [pp.tile([P, TH], F32R, name=f"qtr{c}", tag=f"qtr{c}") for c in range(CH)]
        KTr = [pp.tile([P, T], F32R, name=f"ktr{c}", tag=f"ktr{c}") for c in range(CH)]
        Vr = pp.tile([P, NB, D + 2], F32R, name="vr", tag="vr")  # [V | ones | pad]
        V2r = pp.tile([P, NB, D], F32R, name="v2r", tag="v2r")
        nfcs = pp.tile([P, TH // P, D], F32, name="nfcs", tag="nfcs")
        bqe = [pp.tile([P, 1], F32, name=f"bqe{c}", tag=f"bqe{c}") for c in range(CH)]
        bv_bc = pp.tile([P, D], F32, name="bvbc", tag="bvbc")
        m_bc = pp.tile([P, D], F32, name="mbc", tag="mbc")
        i_bc = pp.tile([P, D], F32, name="ibc", tag="ibc")
        negc0_t = pp.tile([P, 1], F32, name="negc0", tag="negc0")

        # shared stats-chunk pool: one ring zone reserved up front for all
        # three stats pipelines (fs -> fc -> fcs slot cycling, matching the
        # order the data is needed)
        pstat_cm = tc.tile_pool(name="pstat", bufs=1)
        pstat = pstat_cm.__enter__()
        wf_sb = [pstat.tile([P, D], F32, name=f"wf{c}", tag=f"wf{c}") for c in range(CH)]
        wg_sb = [pstat.tile([P, D], F32, name=f"wg{c}", tag=f"wg{c}") for c in range(CH)]
        wh_sb = [pstat.tile([P, D], F32, name=f"wh{c}", tag=f"wh{c}") for c in range(CH)]
        bq_sb = [pstat.tile([P, 1], F32, name=f"bqs{c}", tag=f"bqs{c}") for c in range(CH)]
        for c in range(CH):
            nc.sync.dma_start(out=wh_sb[c], in_=whT[c * P : (c + 1) * P, :])
            nc.sync.dma_start(out=wg_sb[c], in_=wgT[c * P : (c + 1) * P, :])
            nc.sync.dma_start(out=wf_sb[c], in_=wfT[c * P : (c + 1) * P, :])
            nc.sync.dma_start(out=bq_sb[c], in_=bq_e[c * P : (c + 1) * P, :])

        nc.vector.memset(negc0_t, -C0)
        ones_f32 = pstat.tile([P, NB * 2], F32, name="ones32", tag="ones32")
        nc.vector.memset(ones_f32, 1.0)
        nc.vector.tensor_copy(
            Vr[:, :, D : D + 2], ones_f32.rearrange("p (n two) -> p n two", two=2)
        )
        nc.gpsimd.dma_start(out=bv_bc, in_=_bcast_row(bv_e, 0, D))

        scr = pstat.tile([P, CK], F32, name="scr", tag="scr")
        scr2 = pstat.tile([P, CK], F32, name="scr2", tag="scr2")

        def rsqrt_seed(y, v):
            """y = f32 bit-trick rsqrt seed of v (v > 0), via int32 DVE ops."""
            yi, vi = y.bitcast(I32), v.bitcast(I32)
            nc.vector.tensor_scalar(
                yi, vi, 1, 0x7FFFFFFF, op0=OP.logical_shift_right, op1=OP.bitwise_xor
            )
            nc.vector.tensor_scalar(yi, yi, RSQRT_REST, None, op0=OP.subtract)

        def rsqrt_iter(y, v, a, dst=None):
            """One Newton rsqrt step: dst/y = y*(1.5 - 0.5*v*y^2). a is scratch."""
            nc.vector.tensor_mul(a, y, y)
            nc.vector.tensor_mul(a, a, v)
            nc.vector.tensor_scalar(a, a, -0.5, 1.5, op0=OP.mult, op1=OP.add)
            nc.vector.tensor_mul(dst if dst is not None else y, y, a)

        def inv_std(dst, acc_q, mean, name):
            """dst = 1/sqrt(E[x^2] - mean^2 + eps), DVE-only (no ACT sqrt —
            keeps ScalarE on the exp table set for the whole kernel)."""
            v = pstat.tile([P, 1], F32, name=f"{name}v", tag=f"{name}v")
            nc.vector.reduce_sum(v, acc_q, axis=mybir.AxisListType.X)
            nc.vector.tensor_scalar(v, v, 1.0 / T, EPS_IN, op0=OP.mult, op1=OP.add)
            msq = pstat.tile([P, 1], F32, name=f"{name}msq", tag=f"{name}msq")
            nc.vector.tensor_mul(msq, mean, mean)
            nc.vector.tensor_sub(v, v, msq)
            a = pstat.tile([P, 1], F32, name=f"{name}ra", tag=f"{name}ra")
            rsqrt_seed(dst, v)
            for _ in range(3):
                rsqrt_iter(dst, v, a)

        def stats_pass(x_ext, name, round_to=None, round_cols=0, dve_only=False):
            """Per-channel mean/inv_std of a (D,T) DRAM tensor via chunked
            sum + sumsq accumulate passes; the sum pass writes the rounded
            f32r copy used by the projections. Returns (mean, invs) plus a
            per-chunk callback list executed after each (k, c=last) unit."""
            mean = [pp.tile([P, 1], F32, name=f"{name}m{c}", tag=f"{name}m{c}") for c in range(CH)]
            invs = [pp.tile([P, 1], F32, name=f"{name}i{c}", tag=f"{name}i{c}") for c in range(CH)]
            acc_s = [pstat.tile([P, NCK], F32, name=f"{name}as{c}", tag=f"{name}as{c}") for c in range(CH)]
            acc_q = [pstat.tile([P, NCK], F32, name=f"{name}aq{c}", tag=f"{name}aq{c}") for c in range(CH)]

            def unit(k, c):
                ck = pstat.tile([P, CK], F32, name=f"{name}ck{c}_{k}", tag="ck", bufs=2)
                nc.sync.dma_start(
                    out=ck,
                    in_=x_ext[c * P : (c + 1) * P, k * CK : (k + 1) * CK],
                )
                if round_to is not None and (k + 1) * CK <= round_cols:
                    dst = round_to[c][:, k * CK : (k + 1) * CK]
                else:
                    dst = scr
                if dve_only:
                    nc.vector.tensor_scalar(
                        dst, ck, 0.0, 0.0, op0=OP.add, op1=OP.add,
                        accum_out=acc_s[c][:, k : k + 1],
                    )
                    nc.vector.scalar_tensor_tensor(
                        scr2, ck, 0.0, ck, op0=OP.add, op1=OP.mult,
                        accum_out=acc_q[c][:, k : k + 1],
                    )
                elif (k + c) % 2 == 0:
                    nc.scalar.activation(dst, ck, AF.Copy, accum_out=acc_s[c][:, k : k + 1])
                    nc.vector.scalar_tensor_tensor(
                        scr2, ck, 0.0, ck, op0=OP.add, op1=OP.mult,
                        accum_out=acc_q[c][:, k : k + 1],
                    )
                else:
                    nc.vector.tensor_scalar(
                        dst, ck, 0.0, 0.0, op0=OP.add, op1=OP.add,
                        accum_out=acc_s[c][:, k : k + 1],
                    )
                    nc.scalar.activation(
                        scr2, ck, AF.Square, accum_out=acc_q[c][:, k : k + 1]
                    )

            def finalize():
                for c in range(CH):
                    nc.vector.reduce_sum(mean[c], acc_s[c], axis=mybir.AxisListType.X)
                    nc.vector.tensor_scalar_mul(mean[c], mean[c], 1.0 / T)
                    inv_std(invs[c], acc_q[c], mean[c], f"{name}{c}")

            return mean, invs, unit, finalize

        # ---------------- phase fs: stats + V + K projections --------------
        with tc.tile_pool(name="pfs", bufs=1) as pfs, tc.tile_pool(
            name="psv", bufs=2, space="PSUM"
        ) as psv, tc.tile_pool(name="psk", bufs=3, space="PSUM") as psk:
            fsr = [pfs.tile([P, T], F32R, name=f"fsr{c}", tag=f"fsr{c}") for c in range(CH)]
            wk = [pfs.tile([P, D], F32R, name=f"wk{c}", tag=f"wk{c}") for c in range(CH)]
            wv = [pfs.tile([P, D], F32R, name=f"wv{c}", tag=f"wv{c}") for c in range(CH)]
            for c in range(CH):
                nc.vector.tensor_copy(wv[c], wh_sb[c])

            m_s, i_s, fs_unit, fs_fin = stats_pass(fsT, "fs", round_to=fsr, round_cols=T)
            with nc.named_scope("fsv"):
                for k in range(NCK):
                    for c in range(CH):
                        fs_unit(k, c)
                    # V projection for this chunk's 16 tk blocks (V needs no
                    # stats; keeps the PE warm during the stats DMA stream).
                    # Pairs of blocks share one [P,512] PSUM tile / evac.
                    for tp in range(CK // P // 2):
                        pv = psv.tile([P, 512], F32, name=f"pv{k}_{tp}", tag="pv")
                        for h in range(2):
                            tb = k * (CK // P) + tp * 2 + h
                            sl = slice(tb * P, (tb + 1) * P)
                            hs = slice(h * 256, (h + 1) * 256)
                            nc.tensor.matmul(pv[:, hs], fsr[0][:, sl], wv[0], start=True, stop=False)
                            nc.tensor.matmul(pv[:, hs], fsr[1][:, sl], wv[1], start=False, stop=True)
                        t0 = k * (CK // P) + tp * 2
                        dst = Vr[:, t0 : t0 + 2, 0:D]
                        src = pv.rearrange("p (two d) -> p two d", two=2)
                        if tp % 2 == 0:
                            nc.scalar.activation(dst, src, AF.Copy)
                        else:
                            nc.vector.tensor_copy(dst, src)
                        nc.gpsimd.tensor_mul(
                            V2r[:, t0 : t0 + 2, :], Vr[:, t0 : t0 + 2, 0:D],
                            Vr[:, t0 : t0 + 2, 0:D],
                        )
                fs_fin()
            for c in range(CH):
                nc.vector.tensor_scalar_mul(wk[c], wg_sb[c], i_s[c])
            # K^T projection (o, tk) full T; bias and mean correction are
            # per-query constants in the logits -> cancel in softmax.
            with nc.named_scope("kproj"):
                for oc in range(CH):
                    for tch in range(T // 512):
                        pk = psk.tile([P, 512], F32, name=f"pk{oc}_{tch}", tag="pk")
                        sl = slice(tch * 512, (tch + 1) * 512)
                        nc.tensor.matmul(
                            pk, wk[0][:, oc * P : (oc + 1) * P], fsr[0][:, sl],
                            start=True, stop=False,
                        )
                        nc.tensor.matmul(
                            pk, wk[1][:, oc * P : (oc + 1) * P], fsr[1][:, sl],
                            start=False, stop=True,
                        )
                        if tch % 2 == 0:
                            nc.scalar.activation(KTr[oc][:, sl], pk, AF.Copy)
                        else:
                            nc.vector.tensor_copy(KTr[oc][:, sl], pk)

        # ---------------- phase fc: stats + Q projection -------------------
        with tc.tile_pool(name="pfc", bufs=1) as pfc, tc.tile_pool(
            name="psq", bufs=3, space="PSUM"
        ) as psq, tc.tile_pool(name="psb", bufs=2, space="PSUM") as psb:
            fcr = [pfc.tile([P, TH], F32R, name=f"fcr{c}", tag=f"fcr{c}") for c in range(CH)]
            wq = [pfc.tile([P, D], F32R, name=f"wq{c}", tag=f"wq{c}") for c in range(CH)]
            m_c, i_c, fc_unit, fc_fin = stats_pass(fcT, "fc", round_to=fcr, round_cols=TH)
            with nc.named_scope("fcstats"):
                for k in range(NCK):
                    for c in range(CH):
                        fc_unit(k, c)
                fc_fin()
            for c in range(CH):
                nc.vector.tensor_scalar_mul(wq[c], wf_sb[c], i_c[c])
            m_r = [pfc.tile([P, 2], F32R, name=f"fcmr{c}", tag=f"fcmr{c}") for c in range(CH)]
            for c in range(CH):
                nc.vector.tensor_copy(m_r[c], m_c[c].to_broadcast((P, 2)))
            for oc in range(CH):
                pb = psb.tile([P, 2], F32, name=f"pbq{oc}", tag="pbq")
                nc.tensor.matmul(pb, wq[0][:, oc * P : (oc + 1) * P], m_r[0], start=True, stop=False)
                nc.tensor.matmul(pb, wq[1][:, oc * P : (oc + 1) * P], m_r[1], start=False, stop=True)
                nc.vector.tensor_sub(bqe[oc], bq_sb[oc], pb[:, 0:1])
            # Q^T projection: core's own half is host-permuted to cols 0:TH
            with nc.named_scope("qproj"):
                for oc in range(CH):
                    for tch in range(TH // 512):
                        pq = psq.tile([P, 512], F32, name=f"pq{oc}_{tch}", tag="pq")
                        sl = slice(tch * 512, (tch + 1) * 512)
                        nc.tensor.matmul(
                            pq, wq[0][:, oc * P : (oc + 1) * P], fcr[0][:, sl],
                            start=True, stop=False,
                        )
                        nc.tensor.matmul(
                            pq, wq[1][:, oc * P : (oc + 1) * P], fcr[1][:, sl],
                            start=False, stop=True,
                        )
                        if tch % 2 == 0:
                            nc.scalar.activation(QTr[oc][:, sl], pq, AF.Identity, bias=bqe[oc])
                        else:
                            nc.vector.tensor_scalar_add(QTr[oc][:, sl], pq, bqe[oc])

        # ---------------- attention (fcs stats folded in) ------------------
        with tc.tile_pool(name="sts", bufs=3) as sts, tc.tile_pool(
            name="epi", bufs=3
        ) as epi, tc.tile_pool(name="psl", bufs=2, space="PSUM") as psl, tc.tile_pool(
            name="pmv", bufs=1, space="PSUM"
        ) as pmv, tc.tile_pool(name="pv2", bufs=1, space="PSUM") as pv2:
            # fcs stats (DVE-only; ACT is saturated with softmax exp) and the
            # nfcs normalize (GpSimd) -- only epilogues consume these.
            nc.sync.dma_start(
                out=nfcs,
                in_=fcsh[:, :].rearrange("(n p) d -> p n d", p=P),
            )
            m_cs, i_cs, cs_unit, cs_fin = stats_pass(fcsT, "fcs", dve_only=True)
            with nc.named_scope("fcsstats"):
                for k in range(NCK):
                    for c in range(CH):
                        cs_unit(k, c)
                cs_fin()
            for c in range(CH):
                nc.gpsimd.dma_start(out=scm[0, c * P : (c + 1) * P], in_=m_cs[c])
                nc.gpsimd.dma_start(out=scm[1, c * P : (c + 1) * P], in_=i_cs[c])
            nc.gpsimd.dma_start(out=m_bc, in_=_bcast_row(scm, 0, D))
            nc.gpsimd.dma_start(out=i_bc, in_=_bcast_row(scm, D, D))
            for b in range(TH // P):
                nc.gpsimd.tensor_sub(nfcs[:, b, :], nfcs[:, b, :], m_bc)
                nc.gpsimd.tensor_mul(nfcs[:, b, :], nfcs[:, b, :], i_bc)

            for q in range(NQ):  # tq chunks of 256
                qsl = slice(q * 256, (q + 1) * 256)
                mv = [pmv.tile([P, D + 2], F32, name=f"mv{q}_{i}", tag=f"mv{i}") for i in range(2)]
                v2 = [pv2.tile([P, D], F32, name=f"v2_{q}_{i}", tag=f"v2{i}") for i in range(2)]
                sts_tiles = [None] * NG

                def emit_logits(g, q=q, qsl=qsl, sts_tiles=sts_tiles):
                    # 4 tk-blocks of logits^T into one 2-bank PSUM tile, one
                    # [128,1024] exp ACTIVATE into f32r SBUF
                    pl = psl.tile([P, 1024], F32, name=f"pl{q}_{g}", tag="pl")
                    for h in range(4):
                        j = 4 * g + h
                        osl = slice(h * 256, (h + 1) * 256)
                        nc.tensor.matmul(
                            pl[:, osl], KTr[0][:, j * P : (j + 1) * P], QTr[0][:, qsl],
                            start=True, stop=False,
                        )
                        nc.tensor.matmul(
                            pl[:, osl], KTr[1][:, j * P : (j + 1) * P], QTr[1][:, qsl],
                            start=False, stop=True,
                        )
                    st = sts.tile([P, 1024], F32R, name="st", tag="st")
                    nc.scalar.activation(st, pl, AF.Exp, bias=negc0_t)
                    sts_tiles[g] = st

                def emit_av(g, q=q, mv=mv, v2=v2, sts_tiles=sts_tiles):
                    st = sts_tiles[g]
                    for h in range(4):
                        j = 4 * g + h
                        for b in range(2):
                            lhs = st[:, h * 256 + b * P : h * 256 + (b + 1) * P]
                            nc.tensor.matmul(
                                mv[b], lhs, Vr[:, j, :],
                                start=(j == 0), stop=(j == NB - 1),
                            )
                            nc.tensor.matmul(
                                v2[b], lhs, V2r[:, j, :],
                                start=(j == 0), stop=(j == NB - 1),
                            )

                emit_logits(0)
                for g in range(1, NG):
                    emit_logits(g)
                    emit_av(g - 1)
                emit_av(NG - 1)

                for b in range(2):
                    qb = q * 2 + b
                    # evacuate PSUM right away so the next chunk's matmuls
                    # reuse the banks without waiting on the epilogue
                    mve = epi.tile([P, D + 2], F32, name="mve", tag="mve")
                    nc.vector.tensor_copy(mve, mv[b])
                    v2e = epi.tile([P, D], F32, name="v2e", tag="v2e")
                    nc.vector.tensor_copy(v2e, v2[b])
                    recip = epi.tile([P, 1], F32, name="recip", tag="recip")
                    nc.vector.reciprocal(recip, mve[:, D : D + 1])
                    Mt = epi.tile([P, D], F32, name="Mt", tag="Mt")
                    nc.vector.tensor_scalar_mul(Mt, mve[:, 0:D], recip)  # unbiased M
                    Msq = epi.tile([P, D], F32, name="Msq", tag="Msq")
                    nc.gpsimd.tensor_mul(Msq, Mt, Mt)
                    # Var -> v2e (in place), clamp; S = Var*rsqrt(Var) via
                    # bit-trick + 2 Newton steps (no ACT sqrt -> no table
                    # switches away from the exp set).
                    nc.vector.scalar_tensor_tensor(
                        v2e, v2e, recip, Msq, op0=OP.mult, op1=OP.subtract
                    )
                    nc.vector.tensor_scalar_max(v2e, v2e, EPS_VAR)
                    ry = epi.tile([P, D], F32, name="ry", tag="ry")
                    ra = epi.tile([P, D], F32, name="ra", tag="ra")
                    rsqrt_seed(ry, v2e)
                    nc.gpsimd.tensor_mul(ra, ry, ry)
                    nc.gpsimd.tensor_mul(ra, ra, v2e)
                    nc.vector.tensor_scalar(ra, ra, -0.5, 1.5, op0=OP.mult, op1=OP.add)
                    nc.vector.tensor_mul(ry, ry, ra)
                    rsqrt_iter(ry, v2e, ra)
                    # out = Var*rsqrt(Var)*nfcs + M + bh
                    nc.gpsimd.tensor_mul(ry, ry, v2e)
                    nc.vector.tensor_mul(ry, ry, nfcs[:, qb, :])
                    nc.gpsimd.tensor_add(ry, ry, Mt)
                    nc.gpsimd.tensor_add(ry, ry, bv_bc)
                    nc.sync.dma_start(out=out_e[qb * P : (qb + 1) * P, :], in_=ry)

        pstat_cm.__exit__(None, None, None)
        persist.__exit__(None, None, None)

    nc.compile()
    return nc


_CACHE = {}


def _get_nc():
    if "nc" not in _CACHE:
        _CACHE["nc"] = build_nc()
    return _CACHE["nc"]


def kernel(**inputs):
    fc = np.ascontiguousarray(np.asarray(inputs["fc"], dtype=np.float32))
    fs = np.ascontiguousarray(np.asarray(inputs["fs"], dtype=np.float32))
    fcs = np.ascontiguousarray(np.asarray(inputs["fcs"], dtype=np.float32))
    Wf = np.asarray(inputs["Wf"], dtype=np.float32)
    bf = np.asarray(inputs["bf"], dtype=np.float32)
    Wg = np.asarray(inputs["Wg"], dtype=np.float32)
    Wh = np.asarray(inputs["Wh"], dtype=np.float32)
    bh = np.asarray(inputs["bh"], dtype=np.float32)

    wfT = np.ascontiguousarray(Wf.T)
    wgT = np.ascontiguousarray(Wg.T)
    whT = np.ascontiguousarray(Wh.T)
    bq = np.ascontiguousarray(bf.reshape(D, 1))

    in_maps = []
    for core in range(8):
        s, h = divmod(core, 2)
        fcT_s = fc[s].T  # (D, T)
        if h == 0:
            fcT_perm = np.ascontiguousarray(fcT_s)
        else:
            fcT_perm = np.ascontiguousarray(
                np.concatenate([fcT_s[:, TH:], fcT_s[:, :TH]], axis=1)
            )
        in_maps.append(
            {
                "fcT": fcT_perm,
                "fsT": np.ascontiguousarray(fs[s].T),
                "fcsT": np.ascontiguousarray(fcs[s].T),
                "fcsh": np.ascontiguousarray(fcs[s, h * TH : (h + 1) * TH, :]),
                "wfT": wfT,
                "wgT": wgT,
                "whT": whT,
                "bq": bq,
                "bv": bh,
            }
        )

    nc = _get_nc()
    res = run_bass_kernel_spmd(
        nc, in_maps, core_ids=list(range(8)), trace=TRACE, **TRACE_KW
    )
    if TRACE:
        _CACHE["last_result"] = res

    out = np.empty((4, T, D), np.float32)
    for core in range(8):
        s, h = divmod(core, 2)
        out[s, h * TH : (h + 1) * TH, :] = res.results[core]["out"]
    return out


# revision 19
# speedup vs baseline: 1.0585x; 1.0585x over previous
"""AdaAttN attention kernel for 8 TRN2 NeuronCores.

Problem: nn_AdaAttN_29076928593982
  fc, fs, fcs: (4, 4096, 256) f32; Wf/Wg/Wh (256,256); bf/bg/bh (256,)
  Q = Wf@inorm(fc_t)+bf; K = Wg@inorm(fs_t)+bg; V = Wh@fs_t+bh
  A = softmax(Q K); M = A V; Var = A V^2 - M^2; S = sqrt(max(Var,1e-6))
  out = S * inorm(fcs_t) + M   (all in (b, t, d))

Sharding: data-parallel over (sample, query-half): core i -> sample i//2,
query rows [ (i%2)*2048, +2048 ). K/V replicated per sample. No collectives.

Device strategy (v3):
  - all DRAM tensors are host-laid-out so every DMA is a contiguous
    region with >=8KB per-partition lines (the [D,T] column-chunk reads of
    v1/v2 had 16KB-strided 1-8KB lines -> ~100GB/s and descriptor crawl):
      fs/fc/fcs: [NCK, D, CK] pre-chunked; nfcs source: [P, 16*D]
      partition-major; out: [NQ, P, 2*D] pair blocks.
  - two HWDGE rings: c=0 chunk units on nc.sync, c=1 on nc.scalar, so the
    two stats streams descriptor-generate and fetch in parallel. Weights
    and small broadcasts go via gpsimd SWDGE, off the hot rings.
  - ONE ACT table set for the whole kernel ({Exp, Copy, Identity, Square}):
    inv_std and the epilogue sqrt use a bit-trick rsqrt seed + Newton on
    DVE/GpSimd, not ACT Sqrt/Ln. v1 paid ~22 table loads (~59us).
  - softmax exp over [128,1024] tiles (4 tk-blocks per ACTIVATE, 2-bank
    PSUM logit tiles) to amortize the 352-cycle ACT fixed overhead.
  - K's bias AND instance-norm mean correction are dropped: both are
    per-query logit constants, which cancel exactly in softmax.
  - V projection is interleaved into the fs stats stream (V needs no
    stats), warming the PE early; K/Q follow; attention starts ~30us in.
  - stats sum-pass doubles as the f32r rounding pass; dead stats outputs
    are written in place over the DMA chunk (no scratch tiles).
  - fcs stats run DVE-only; the epilogue is split: stage A (PSUM evac,
    Var, rsqrt) runs inline, stage B (nfcs normalize + final combine +
    store) is deferred one chunk so the fcs-stats broadcast dependency
    never head-of-line-blocks an engine FIFO.
"""
import sys

sys.path.insert(0, "/opt/trn_rl_repo")

import numpy as np

import concourse.bass as bass
import concourse.tile as tile
from concourse import bacc
from concourse import mybir
from concourse.bass_utils import run_bass_kernel_spmd

F32 = mybir.dt.float32
F32R = mybir.dt.float32r
I32 = mybir.dt.int32
AF = mybir.ActivationFunctionType
OP = mybir.AluOpType

P = 128          # partitions
D = 256          # feature dim
T = 4096         # tokens per sample
TH = 2048        # query tokens per core
CH = 2           # channel chunks (D // P)
NB = T // P      # tk blocks (32)
NG = NB // 4     # tk groups of 4 blocks (8)
NQ = TH // 256   # tq chunks of 256 (8)
C0 = 110.0       # global softmax shift
EPS_IN = 1e-5
EPS_VAR = 1e-6
CK = 2048        # stats chunk width (1 MiB contiguous per unit)
NCK = T // CK    # 2

# rsqrt bit-trick: MAGIC - (bits>>1) == ((bits>>1) ^ 0x7FFFFFFF) - REST
RSQRT_MAGIC = 0x5F3759DF
RSQRT_REST = 0x7FFFFFFF - RSQRT_MAGIC

TRACE = False    # test.py sets this to get exec_time_ns
TRACE_KW = {}


def _bcast_row(handle, offset, n):
    """AP reading a DRAM row of n elements broadcast across 128 partitions."""
    return bass.AP(tensor=handle, offset=offset, ap=[[0, P], [1, n]])


def build_nc():
    nc = bacc.Bacc()

    fsT = nc.declare_dram_parameter("fsT", [NCK, D, CK], F32, isOutput=False)
    fcT = nc.declare_dram_parameter("fcT", [NCK, D, CK], F32, isOutput=False)
    fcsT = nc.declare_dram_parameter("fcsT", [NCK, D, CK], F32, isOutput=False)
    fcsh = nc.declare_dram_parameter("fcsh", [P, (TH // P) * D], F32, isOutput=False)
    wfT = nc.declare_dram_parameter("wfT", [D, D], F32, isOutput=False)
    wgT = nc.declare_dram_parameter("wgT", [D, D], F32, isOutput=False)
    whT = nc.declare_dram_parameter("whT", [D, D], F32, isOutput=False)
    bq_e = nc.declare_dram_parameter("bq", [D, 1], F32, isOutput=False)
    bv_e = nc.declare_dram_parameter("bv", [D], F32, isOutput=False)
    out_e = nc.declare_dram_parameter("out", [NQ, P, 2 * D], F32, isOutput=True)

    scm = nc.dram_tensor("scm", [2, D], F32)  # fcs stats roundtrip scratch

    with tile.TileContext(nc) as tc:
        persist = tc.tile_pool(name="persist", bufs=1)
        pp = persist.__enter__()

        QTr = [pp.tile([P, TH], F32R, name=f"qtr{c}", tag=f"qtr{c}") for c in range(CH)]
        KTr = [pp.tile([P, T], F32R, name=f"ktr{c}", tag=f"ktr{c}") for c in range(CH)]
        Vr = pp.tile([P, NB, D + 2], F32R, name="vr", tag="vr")  # [V | ones | pad]
        V2r = pp.tile([P, NB, D], F32R, name="v2r", tag="v2r")
        bqe = [pp.tile([P, 1], F32, name=f"bqe{c}", tag=f"bqe{c}") for c in range(CH)]
        bv_bc = pp.tile([P, D], F32, name="bvbc", tag="bvbc")
        m_bc = pp.tile([P, D], F32, name="mbc", tag="mbc")
        i_bc = pp.tile([P, D], F32, name="ibc", tag="ibc")
        negc0_t = pp.tile([P, 1], F32, name="negc0", tag="negc0")

        pstat_cm = tc.tile_pool(name="pstat", bufs=1)
        pstat = pstat_cm.__enter__()
        wf_sb = [pstat.tile([P, D], F32, name=f"wf{c}", tag=f"wf{c}") for c in range(CH)]
        wg_sb = [pstat.tile([P, D], F32, name=f"wg{c}", tag=f"wg{c}") for c in range(CH)]
        wh_sb = [pstat.tile([P, D], F32, name=f"wh{c}", tag=f"wh{c}") for c in range(CH)]
        bq_sb = [pstat.tile([P, 1], F32, name=f"bqs{c}", tag=f"bqs{c}") for c in range(CH)]
        # weights/bias via gpsimd SWDGE -- keeps the two HWDGE rings free
        # for the big streaming loads
        for c in range(CH):
            nc.gpsimd.dma_start(out=wh_sb[c], in_=whT[c * P : (c + 1) * P, :])
            nc.gpsimd.dma_start(out=wg_sb[c], in_=wgT[c * P : (c + 1) * P, :])
            nc.gpsimd.dma_start(out=wf_sb[c], in_=wfT[c * P : (c + 1) * P, :])
            nc.gpsimd.dma_start(out=bq_sb[c], in_=bq_e[c * P : (c + 1) * P, :])
        nc.gpsimd.dma_start(out=bv_bc, in_=_bcast_row(bv_e, 0, D))

        nc.vector.memset(negc0_t, -C0)
        ones_f32 = pstat.tile([P, NB * 2], F32, name="ones32", tag="ones32")
        nc.vector.memset(ones_f32, 1.0)
        nc.vector.tensor_copy(
            Vr[:, :, D : D + 2], ones_f32.rearrange("p (n two) -> p n two", two=2)
        )
        # shared dead-output scratch for the fs/fc square passes (lets the
        # sum and square passes of one chunk run on both engines in parallel)
        scr2 = pstat.tile([P, CK], F32, name="scr2", tag="scr2")

        def rsqrt_seed(y, v):
            """y = f32 bit-trick rsqrt seed of v (v > 0), via int32 DVE ops."""
            yi, vi = y.bitcast(I32), v.bitcast(I32)
            nc.vector.tensor_scalar(
                yi, vi, 1, 0x7FFFFFFF, op0=OP.logical_shift_right, op1=OP.bitwise_xor
            )
            nc.vector.tensor_scalar(yi, yi, RSQRT_REST, None, op0=OP.subtract)

        def rsqrt_iter(y, v, a):
            """One Newton rsqrt step on DVE: y *= 1.5 - 0.5*v*y^2 (a scratch)."""
            nc.vector.tensor_mul(a, y, y)
            nc.vector.scalar_tensor_tensor(a, a, -0.5, v, op0=OP.mult, op1=OP.mult)
            nc.vector.scalar_tensor_tensor(y, a, 1.5, y, op0=OP.add, op1=OP.mult)

        def inv_std(dst, acc_q, mean, name):
            """dst = 1/sqrt(E[x^2] - mean^2 + eps), DVE-only."""
            v = pstat.tile([P, 1], F32, name=f"{name}v", tag=f"{name}v")
            nc.vector.reduce_sum(v, acc_q, axis=mybir.AxisListType.X)
            nc.vector.tensor_scalar(v, v, 1.0 / T, EPS_IN, op0=OP.mult, op1=OP.add)
            msq = pstat.tile([P, 1], F32, name=f"{name}msq", tag=f"{name}msq")
            nc.vector.tensor_mul(msq, mean, mean)
            nc.vector.tensor_sub(v, v, msq)
            a = pstat.tile([P, 1], F32, name=f"{name}ra", tag=f"{name}ra")
            rsqrt_seed(dst, v)
            for _ in range(3):
                rsqrt_iter(dst, v, a)

        def stats_pass(x_ext, name, round_to=None, round_cols=0, dve_only=False):
            """Per-channel mean/inv_std of a pre-chunked [NCK, D, CK] DRAM
            tensor. The sum pass writes the f32r rounded copy (or rounds the
            chunk in place when the copy isn't needed); the square pass then
            squares the chunk in place. c=0 units load on the sync HWDGE
            ring, c=1 on the scalar ring."""
            mean = [pp.tile([P, 1], F32, name=f"{name}m{c}", tag=f"{name}m{c}") for c in range(CH)]
            invs = [pp.tile([P, 1], F32, name=f"{name}i{c}", tag=f"{name}i{c}") for c in range(CH)]
            acc_s = [pstat.tile([P, NCK], F32, name=f"{name}as{c}", tag=f"{name}as{c}") for c in range(CH)]
            acc_q = [pstat.tile([P, NCK], F32, name=f"{name}aq{c}", tag=f"{name}aq{c}") for c in range(CH)]

            def unit(k, c):
                ck = pstat.tile([P, CK], F32, name=f"{name}ck{c}_{k}", tag="ck", bufs=3)
                eng = nc.sync if c == 0 else nc.scalar
                eng.dma_start(out=ck, in_=x_ext[k, c * P : (c + 1) * P, :])
                if round_to is not None and (k + 1) * CK <= round_cols:
                    dst = round_to[c][:, k * CK : (k + 1) * CK]
                else:
                    dst = ck[:, :].bitcast(F32R)  # dead sum output: round in place
                if dve_only:
                    nc.vector.tensor_scalar(
                        dst, ck, 0.0, 0.0, op0=OP.add, op1=OP.add,
                        accum_out=acc_s[c][:, k : k + 1],
                    )
                    nc.vector.scalar_tensor_tensor(
                        ck, ck, 0.0, ck, op0=OP.add, op1=OP.mult,
                        accum_out=acc_q[c][:, k : k + 1],
                    )
                elif (k + c) % 2 == 0:
                    nc.scalar.activation(dst, ck, AF.Copy, accum_out=acc_s[c][:, k : k + 1])
                    nc.vector.scalar_tensor_tensor(
                        scr2, ck, 0.0, ck, op0=OP.add, op1=OP.mult,
                        accum_out=acc_q[c][:, k : k + 1],
                    )
                else:
                    nc.vector.tensor_scalar(
                        dst, ck, 0.0, 0.0, op0=OP.add, op1=OP.add,
                        accum_out=acc_s[c][:, k : k + 1],
                    )
                    nc.scalar.activation(
                        scr2, ck, AF.Square, accum_out=acc_q[c][:, k : k + 1]
                    )

            def finalize():
                for c in range(CH):
                    nc.vector.reduce_sum(mean[c], acc_s[c], axis=mybir.AxisListType.X)
                    nc.vector.tensor_scalar_mul(mean[c], mean[c], 1.0 / T)
                    inv_std(invs[c], acc_q[c], mean[c], f"{name}{c}")

            return mean, invs, unit, finalize

        # ---------------- phase fs: stats + V + K projections --------------
        with tc.tile_pool(name="pfs", bufs=1) as pfs, tc.tile_pool(
            name="psv", bufs=2, space="PSUM"
        ) as psv, tc.tile_pool(name="psk", bufs=3, space="PSUM") as psk:
            fsr = [pfs.tile([P, T], F32R, name=f"fsr{c}", tag=f"fsr{c}") for c in range(CH)]
            wk = [pfs.tile([P, D], F32R, name=f"wk{c}", tag=f"wk{c}") for c in range(CH)]
            wv = [pfs.tile([P, D], F32R, name=f"wv{c}", tag=f"wv{c}") for c in range(CH)]
            for c in range(CH):
                nc.vector.tensor_copy(wv[c], wh_sb[c])

            m_s, i_s, fs_unit, fs_fin = stats_pass(fsT, "fs", round_to=fsr, round_cols=T)
            with nc.named_scope("fsv"):
                for k in range(NCK):
                    for c in range(CH):
                        fs_unit(k, c)
                    # V projection for this chunk's 16 tk blocks; block pairs
                    # share one [P,512] PSUM tile / evac copy.
                    for tp in range(CK // P // 2):
                        pv = psv.tile([P, 512], F32, name=f"pv{k}_{tp}", tag="pv")
                        for h in range(2):
                            tb = k * (CK // P) + tp * 2 + h
                            sl = slice(tb * P, (tb + 1) * P)
                            hs = slice(h * 256, (h + 1) * 256)
                            nc.tensor.matmul(pv[:, hs], fsr[0][:, sl], wv[0], start=True, stop=False)
                            nc.tensor.matmul(pv[:, hs], fsr[1][:, sl], wv[1], start=False, stop=True)
                        t0 = k * (CK // P) + tp * 2
                        dst = Vr[:, t0 : t0 + 2, 0:D]
                        src = pv.rearrange("p (two d) -> p two d", two=2)
                        if tp % 2 == 0:
                            nc.scalar.activation(dst, src, AF.Copy)
                        else:
                            nc.vector.tensor_copy(dst, src)
                        nc.gpsimd.tensor_mul(
                            V2r[:, t0 : t0 + 2, :], Vr[:, t0 : t0 + 2, 0:D],
                            Vr[:, t0 : t0 + 2, 0:D],
                        )
                fs_fin()
            for c in range(CH):
                nc.vector.tensor_scalar_mul(wk[c], wg_sb[c], i_s[c])
            # K^T projection; bias/mean corrections cancel in softmax.
            with nc.named_scope("kproj"):
                for oc in range(CH):
                    for tch in range(T // 512):
                        pk = psk.tile([P, 512], F32, name=f"pk{oc}_{tch}", tag="pk")
                        sl = slice(tch * 512, (tch + 1) * 512)
                        nc.tensor.matmul(
                            pk, wk[0][:, oc * P : (oc + 1) * P], fsr[0][:, sl],
                            start=True, stop=False,
                        )
                        nc.tensor.matmul(
                            pk, wk[1][:, oc * P : (oc + 1) * P], fsr[1][:, sl],
                            start=False, stop=True,
                        )
                        if tch % 2 == 0:
                            nc.scalar.activation(KTr[oc][:, sl], pk, AF.Copy)
                        else:
                            nc.vector.tensor_copy(KTr[oc][:, sl], pk)

        # ---------------- phase fc: stats + Q projection -------------------
        with tc.tile_pool(name="pfc", bufs=1) as pfc, tc.tile_pool(
            name="psq", bufs=3, space="PSUM"
        ) as psq, tc.tile_pool(name="psb", bufs=2, space="PSUM") as psb:
            fcr = [pfc.tile([P, TH], F32R, name=f"fcr{c}", tag=f"fcr{c}") for c in range(CH)]
            wq = [pfc.tile([P, D], F32R, name=f"wq{c}", tag=f"wq{c}") for c in range(CH)]
            m_c, i_c, fc_unit, fc_fin = stats_pass(fcT, "fc", round_to=fcr, round_cols=TH)
            with nc.named_scope("fcstats"):
                for k in range(NCK):
                    for c in range(CH):
                        fc_unit(k, c)
                fc_fin()
            for c in range(CH):
                nc.vector.tensor_scalar_mul(wq[c], wf_sb[c], i_c[c])
            m_r = [pfc.tile([P, 2], F32R, name=f"fcmr{c}", tag=f"fcmr{c}") for c in range(CH)]
            for c in range(CH):
                nc.vector.tensor_copy(m_r[c], m_c[c].to_broadcast((P, 2)))
            for oc in range(CH):
                pb = psb.tile([P, 2], F32, name=f"pbq{oc}", tag="pbq")
                nc.tensor.matmul(pb, wq[0][:, oc * P : (oc + 1) * P], m_r[0], start=True, stop=False)
                nc.tensor.matmul(pb, wq[1][:, oc * P : (oc + 1) * P], m_r[1], start=False, stop=True)
                nc.vector.tensor_sub(bqe[oc], bq_sb[oc], pb[:, 0:1])
            # Q^T projection: core's own half is host-permuted to cols 0:TH
            with nc.named_scope("qproj"):
                for oc in range(CH):
                    for tch in range(TH // 512):
                        pq = psq.tile([P, 512], F32, name=f"pq{oc}_{tch}", tag="pq")
                        sl = slice(tch * 512, (tch + 1) * 512)
                        nc.tensor.matmul(
                            pq, wq[0][:, oc * P : (oc + 1) * P], fcr[0][:, sl],
                            start=True, stop=False,
                        )
                        nc.tensor.matmul(
                            pq, wq[1][:, oc * P : (oc + 1) * P], fcr[1][:, sl],
                            start=False, stop=True,
                        )
                        if tch % 2 == 0:
                            nc.scalar.activation(QTr[oc][:, sl], pq, AF.Identity, bias=bqe[oc])
                        else:
                            nc.vector.tensor_scalar_add(QTr[oc][:, sl], pq, bqe[oc])

        # ---------------- attention (fcs stats folded in) ------------------
        with tc.tile_pool(name="att", bufs=1) as att, tc.tile_pool(
            name="sts", bufs=2
        ) as sts, tc.tile_pool(name="epi", bufs=2) as epi, tc.tile_pool(
            name="psl", bufs=2, space="PSUM"
        ) as psl, tc.tile_pool(name="pmv", bufs=1, space="PSUM") as pmv, tc.tile_pool(
            name="pv2", bufs=1, space="PSUM"
        ) as pv2:
            nfcs = att.tile([P, TH // P, D], F32, name="nfcs", tag="nfcs")
            nc.scalar.dma_start(
                out=nfcs, in_=fcsh[:, :].rearrange("p (n d) -> p n d", d=D)
            )
            m_cs, i_cs, cs_unit, cs_fin = stats_pass(fcsT, "fcs", dve_only=True)
            with nc.named_scope("fcsstats"):
                for k in range(NCK):
                    for c in range(CH):
                        cs_unit(k, c)
                cs_fin()
            for c in range(CH):
                nc.gpsimd.dma_start(out=scm[0, c * P : (c + 1) * P], in_=m_cs[c])
                nc.gpsimd.dma_start(out=scm[1, c * P : (c + 1) * P], in_=i_cs[c])
            nc.gpsimd.dma_start(out=m_bc, in_=_bcast_row(scm, 0, D))
            nc.gpsimd.dma_start(out=i_bc, in_=_bcast_row(scm, D, D))

            def stage_a(mv_b, v2_b):
                """PSUM evac + denominator/mean/Var + rsqrt — everything
                with no fcs/m_bc dependency. Returns (S, Mt) tiles."""
                mve = epi.tile([P, D + 2], F32, name="mve", tag="mve")
                nc.vector.tensor_copy(mve, mv_b)
                v2e = epi.tile([P, D], F32, name="v2e", tag="v2e")
                nc.vector.tensor_copy(v2e, v2_b)
                recip = epi.tile([P, 1], F32, name="recip", tag="recip")
                nc.vector.reciprocal(recip, mve[:, D : D + 1])
                Mt = epi.tile([P, D], F32, name="Mt", tag="Mt", bufs=4)
                nc.vector.tensor_scalar_mul(Mt, mve[:, 0:D], recip)  # unbiased M
                Msq = epi.tile([P, D], F32, name="Msq", tag="Msq")
                nc.gpsimd.tensor_mul(Msq, Mt, Mt)
                # Var -> v2e (in place), clamp; rsqrt via bit-trick seed +
                # 1 Newton step (max rel err ~2e-3), no ACT table switch.
                nc.vector.scalar_tensor_tensor(
                    v2e, v2e, recip, Msq, op0=OP.mult, op1=OP.subtract
                )
                nc.vector.tensor_scalar_max(v2e, v2e, EPS_VAR)
                ry = epi.tile([P, D], F32, name="ry", tag="ry", bufs=4)
                ra = epi.tile([P, D], F32, name="ra", tag="ra")
                rsqrt_seed(ry, v2e)
                rsqrt_iter(ry, v2e, ra)
                nc.gpsimd.tensor_mul(ry, ry, v2e)  # S = Var*rsqrt(Var)
                return ry, Mt

            def stage_b(q, pend):
                """Deferred: nfcs normalize + out = S*nfcs + M + bh + store.
                Runs one chunk late so the m_bc/i_bc (fcs stats) dependency
                never head-of-line-blocks the GpSimd/DVE FIFOs."""
                outp = epi.tile([P, 2, D], F32, name="outp", tag="outp")
                for b, (S, Mt) in enumerate(pend):
                    qb = q * 2 + b
                    nc.gpsimd.tensor_sub(nfcs[:, qb, :], nfcs[:, qb, :], m_bc)
                    nc.gpsimd.tensor_mul(nfcs[:, qb, :], nfcs[:, qb, :], i_bc)
                    nc.vector.tensor_mul(S, S, nfcs[:, qb, :])
                    nc.gpsimd.tensor_add(S, S, Mt)
                    nc.gpsimd.tensor_add(outp[:, b, :], S, bv_bc)
                nc.sync.dma_start(
                    out=out_e[q], in_=outp.rearrange("p n d -> p (n d)")
                )

            pend_prev = None
            for q in range(NQ):  # tq chunks of 256
                qsl = slice(q * 256, (q + 1) * 256)
                mv = [pmv.tile([P, D + 2], F32, name=f"mv{q}_{i}", tag=f"mv{i}") for i in range(2)]
                v2 = [pv2.tile([P, D], F32, name=f"v2_{q}_{i}", tag=f"v2{i}") for i in range(2)]
                sts_tiles = [None] * NG

                def emit_logits(g, q=q, qsl=qsl, sts_tiles=sts_tiles):
                    # 4 tk-blocks of logits^T into one 2-bank PSUM tile, one
                    # [128,1024] exp ACTIVATE into f32r SBUF
                    pl = psl.tile([P, 1024], F32, name=f"pl{q}_{g}", tag="pl")
                    for h in range(4):
                        j = 4 * g + h
                        osl = slice(h * 256, (h + 1) * 256)
                        nc.tensor.matmul(
                            pl[:, osl], KTr[0][:, j * P : (j + 1) * P], QTr[0][:, qsl],
                            start=True, stop=False,
                        )
                        nc.tensor.matmul(
                            pl[:, osl], KTr[1][:, j * P : (j + 1) * P], QTr[1][:, qsl],
                            start=False, stop=True,
                        )
                    st = sts.tile([P, 1024], F32R, name="st", tag="st")
                    nc.scalar.activation(st, pl, AF.Exp, bias=negc0_t)
                    sts_tiles[g] = st

                def emit_av(g, q=q, mv=mv, v2=v2, sts_tiles=sts_tiles):
                    st = sts_tiles[g]
                    for h in range(4):
                        j = 4 * g + h
                        for b in range(2):
                            lhs = st[:, h * 256 + b * P : h * 256 + (b + 1) * P]
                            nc.tensor.matmul(
                                mv[b], lhs, Vr[:, j, :],
                                start=(j == 0), stop=(j == NB - 1),
                            )
                            nc.tensor.matmul(
                                v2[b], lhs, V2r[:, j, :],
                                start=(j == 0), stop=(j == NB - 1),
                            )

                emit_logits(0)
                for g in range(1, NG):
                    emit_logits(g)
                    emit_av(g - 1)
                emit_av(NG - 1)

                if pend_prev is not None:
                    stage_b(q - 1, pend_prev)
                pend_prev = [stage_a(mv[b], v2[b]) for b in range(2)]
            stage_b(NQ - 1, pend_prev)

        pstat_cm.__exit__(None, None, None)
        persist.__exit__(None, None, None)

    nc.compile()
    return nc


_CACHE = {}


def _get_nc():
    if "nc" not in _CACHE:
        _CACHE["nc"] = build_nc()
    return _CACHE["nc"]


def kernel(**inputs):
    fc = np.ascontiguousarray(np.asarray(inputs["fc"], dtype=np.float32))
    fs = np.ascontiguousarray(np.asarray(inputs["fs"], dtype=np.float32))
    fcs = np.ascontiguousarray(np.asarray(inputs["fcs"], dtype=np.float32))
    Wf = np.asarray(inputs["Wf"], dtype=np.float32)
    bf = np.asarray(inputs["bf"], dtype=np.float32)
    Wg = np.asarray(inputs["Wg"], dtype=np.float32)
    Wh = np.asarray(inputs["Wh"], dtype=np.float32)
    bh = np.asarray(inputs["bh"], dtype=np.float32)

    wfT = np.ascontiguousarray(Wf.T)
    wgT = np.ascontiguousarray(Wg.T)
    whT = np.ascontiguousarray(Wh.T)
    bq = np.ascontiguousarray(bf.reshape(D, 1))

    def chunked(xT):  # (D, T) -> contiguous [NCK, D, CK]
        return np.ascontiguousarray(
            np.stack([xT[:, k * CK : (k + 1) * CK] for k in range(NCK)])
        )

    in_maps = []
    for core in range(8):
        s, h = divmod(core, 2)
        fcT_s = fc[s].T  # (D, T)
        if h == 0:
            fcT_perm = fcT_s
        else:
            fcT_perm = np.concatenate([fcT_s[:, TH:], fcT_s[:, :TH]], axis=1)
        # fcs half in partition-major [P, 16*D] layout for the nfcs load
        fcsh = fcs[s, h * TH : (h + 1) * TH, :].reshape(TH // P, P, D)
        fcsh_p = np.ascontiguousarray(
            fcsh.transpose(1, 0, 2).reshape(P, (TH // P) * D)
        )
        in_maps.append(
            {
                "fcT": chunked(fcT_perm),
                "fsT": chunked(fs[s].T),
                "fcsT": chunked(fcs[s].T),
                "fcsh": fcsh_p,
                "wfT": wfT,
                "wgT": wgT,
                "whT": whT,
                "bq": bq,
                "bv": bh,
            }
        )

    nc = _get_nc()
    res = run_bass_kernel_spmd(
        nc, in_maps, core_ids=list(range(8)), trace=TRACE, **TRACE_KW
    )
    if TRACE:
        _CACHE["last_result"] = res

    out = np.empty((4, T, D), np.float32)
    for core in range(8):
        s, h = divmod(core, 2)
        o = res.results[core]["out"]  # [NQ, P, 2*D]
        o = o.reshape(NQ, P, 2, D).transpose(0, 2, 1, 3).reshape(TH, D)
        out[s, h * TH : (h + 1) * TH, :] = o
    return out
